# revision 1
# baseline (speedup 1.0000x reference)
"""MultiHeadAttention TRN2 kernel: batch-parallel across 8 NeuronCores.

Layout notes (per core, one batch element):
  xT   [768, 1024]  fp16  x[b] as (c, i) -- i = h*32+w token index
  wqk  [768, 1536]  fp16  permuted w_qkv columns: [q_h0 d0..63 (x8 scale), ...,
                          q_h11, k_h0, ..., k_h11]
  wv   [768, 768]   fp16  permuted v columns per head
  wp   [768, 768]   fp16  w_proj
  out  [768, 1024]  fp32  out^T (c', i) == (c, h, w) layout directly

Pipeline: qkT = wqk^T x (PE) -> per head: S = q^T(i-tile) k (PE, K=64) ->
rowmax (DVE) -> exp+rowsum (ACT) -> 1/l (DVE) -> P*1/l (GPSIMD) ->
PE-transpose P -> ctx^T = V^T P^T (PE) -> proj out^T = wp^T ctx^T (PE).
"""
import numpy as np

HEADS, DH, DIM, N = 12, 64, 768, 1024
NB = 8  # batch == cores

_cache = {}


def _fix_drain_waits(nc, mybir, bass_rust):
    """This container's walrus has tight per-instruction sync-wait budgets
    (InstDrain tolerates none; Matmult only a couple). Hoist excess waits
    onto standalone event-semaphore wait instructions placed just before,
    chunked 4 waits apiece."""
    n = 0
    for f in nc.m.functions:
        for bb in f.blocks:
            new = []
            for ins in bb.instructions:
                si = ins.sync_info
                waits = list(si.on_wait) if si and si.on_wait else []
                limit = 0 if isinstance(ins, mybir.InstDrain) else 1
                if isinstance(ins, mybir.InstEventSemaphore):
                    limit = 2
                if len(waits) > limit:
                    keep, excess = waits[:limit], waits[limit:]
                    for c in range(0, len(excess), 2):
                        n += 1
                        ev = mybir.InstEventSemaphore(
                            name=f"{ins.name}-hoistw{n}", ins=[], outs=[])
                        ev.engine = ins.engine
                        ev.sync_info = bass_rust.SyncInfo(
                            on_wait=excess[c:c + 2], on_update=[])
                        nc.register_instruction(ev, overwrite=True)
                        new.append(ev)
                    si.on_wait = keep
                new.append(ins)
            bb.instructions[:] = new
    return n


def _build():
    import sys
    if "/opt/trn_rl_repo" not in sys.path:
        sys.path.insert(0, "/opt/trn_rl_repo")
    import bass_rust
    import concourse.bass as bass
    import concourse.mybir as mybir
    import concourse.tile as tile
    from concourse.masks import make_identity

    FP16, FP32 = mybir.dt.float16, mybir.dt.float32
    AX = mybir.AxisListType.X
    EXP = mybir.ActivationFunctionType.Exp
    KT = DIM // 128  # 6 contraction tiles

    nc = bass.Bass()
    xT = nc.declare_dram_parameter("xT", [DIM, N], FP16, isOutput=False)
    wqk = nc.declare_dram_parameter("wqk", [DIM, 2 * DIM], FP16, isOutput=False)
    wv = nc.declare_dram_parameter("wv", [DIM, DIM], FP16, isOutput=False)
    wp = nc.declare_dram_parameter("wp", [DIM, DIM], FP16, isOutput=False)
    out = nc.declare_dram_parameter("out", [DIM, N], FP32, isOutput=True)

    with tile.TileContext(nc) as tc:
        with (
            tc.tile_pool(name="win", bufs=1) as win,
            tc.tile_pool(name="qk", bufs=1) as qkp,
            tc.tile_pool(name="vp", bufs=1) as vp,
            tc.tile_pool(name="pp", bufs=4) as pp,
            tc.tile_pool(name="pt", bufs=3) as ptp,
            tc.tile_pool(name="st", bufs=8) as st,
            tc.tile_pool(name="cx", bufs=1) as cxp,
            tc.tile_pool(name="ou", bufs=2) as oup,
            tc.tile_pool(name="ps_mm", bufs=3, space="PSUM") as ps_mm,
            tc.tile_pool(name="ps_cx", bufs=1, space="PSUM") as ps_cx,
        ):
            # ---- resident loads
            xsb, wqksb, wvsb, wpsb = [], [], [], []
            for t in range(KT):
                xt = win.tile([128, N], FP16, tag=f"x{t}", name=f"x{t}")
                nc.sync.dma_start(xt[:], xT[t * 128:(t + 1) * 128, :])
                xsb.append(xt)
                wt = win.tile([128, 2 * DIM], FP16, tag=f"wqk{t}", name=f"wqk{t}")
                nc.sync.dma_start(wt[:], wqk[t * 128:(t + 1) * 128, :])
                wqksb.append(wt)
            for t in range(KT):
                vt = win.tile([128, DIM], FP16, tag=f"wv{t}", name=f"wv{t}")
                nc.sync.dma_start(vt[:], wv[t * 128:(t + 1) * 128, :])
                wvsb.append(vt)
                pt_ = win.tile([128, DIM], FP16, tag=f"wp{t}", name=f"wp{t}")
                nc.sync.dma_start(pt_[:], wp[t * 128:(t + 1) * 128, :])
                wpsb.append(pt_)
            idt = win.tile([128, 128], FP16, tag="idt", name="idt")
            make_identity(nc, idt[:])

            HALVES = ((0, 512), (512, 1024))

            # ---- QK^T projection: rows m*128.. of [q^T; k^T] (1536, 1024)
            # order pairs (q-tile, k-tile) per head pair so heads unblock early
            qksb = [None] * 12

            def emit_qk_block(m):
                ps = ps_mm.tile([128, N], FP32, tag="mm", name="mm")
                for t in range(KT):
                    for lo, hi in HALVES:
                        nc.tensor.matmul(
                            ps[:, lo:hi],
                            wqksb[t][:, m * 128:(m + 1) * 128],
                            xsb[t][:, lo:hi],
                            start=(t == 0), stop=(t == KT - 1),
                        )
                qt = qkp.tile([128, N], FP16, tag=f"qk{m}", name=f"qk{m}")
                nc.scalar.copy(qt[:], ps[:])
                qksb[m] = qt

            # ---- V projection: V [1024, 768] j-tiles (tokens on partitions),
            # stored per head with a ones column appended (65 cols/head) so the
            # ctx matmul also produces the softmax row-sums.
            vsb = [None] * 8

            def emit_v_tile(j):
                ps = ps_mm.tile([128, DIM], FP32, tag="mm", name="mm")
                for t in range(KT):
                    nc.tensor.matmul(ps[:, 0:512], xsb[t][:, j * 128:(j + 1) * 128],
                                     wvsb[t][:, 0:512],
                                     start=(t == 0), stop=(t == KT - 1))
                    nc.tensor.matmul(ps[:, 512:768], xsb[t][:, j * 128:(j + 1) * 128],
                                     wvsb[t][:, 512:768],
                                     start=(t == 0), stop=(t == KT - 1))
                vt = vp.tile([128, HEADS * (DH + 1)], FP16, tag=f"v{j}", name=f"v{j}")
                nc.gpsimd.memset(vt[:], 1.0)
                nc.scalar.copy(
                    vt[:, :].rearrange("p (h c) -> p h c", c=DH + 1)[:, :, 0:DH],
                    ps[:, 0:DIM].rearrange("p (h c) -> p h c", c=DH))
                vsb[j] = vt

            ctxall = [cxp.tile([128, DIM], FP16, tag=f"ca{it}", name=f"ca{it}")
                      for it in range(8)]

            # ---- attention heads: instruction-level interleave so the PE
            # queue always has independent work while softmax/DMA chase.
            p_of = {}    # h -> list of 8 P tiles
            ptb_of = {}  # h -> PTbig tile (transposed P)
            psc_of = {}  # h -> ctx^T psum
            cxt_of = {}  # h -> ctx^T sbuf fp16

            def emit_S_item(h, it):
                prow = (h % 2) * 64
                q_ap = qksb[h // 2][prow:prow + 64, :]
                k_ap = qksb[6 + h // 2][prow:prow + 64, :]
                ps = ps_mm.tile([128, N], FP32, tag="mm", name="mm")
                for lo, hi in HALVES:
                    nc.tensor.matmul(ps[:, lo:hi],
                                     q_ap[:, it * 128:(it + 1) * 128],
                                     k_ap[:, lo:hi], start=True, stop=True)
                negmax = st.tile([128, 1], FP32, tag="negmax", name="negmax")
                nc.vector.tensor_reduce(negmax[:], ps[:], axis=AX,
                                        op=mybir.AluOpType.max, negate=True)
                p = pp.tile([128, N], FP16, tag="p", name="p")
                nc.scalar.activation(p[:], ps[:], EXP, bias=negmax[:], scale=1.0)
                p_of.setdefault(h, []).append(p)
                # contiguous-output transpose: ptbI[jp, b*128+c] = P[c, b*128+jp]
                ptbI = ptp.tile([128, N], FP16, tag=f"ptb{it}", name=f"ptb{it}")
                nc.sync.dma_start_transpose(
                    ptbI[:, :].rearrange("p (b c) -> p b c", c=128), p[:])
                ptb_of.setdefault(h, []).append(ptbI)

            def emit_ctx_mm(h, j):
                if j == 0:
                    psc_of[h] = ps_cx.tile([DH + 1, N], FP32, tag="cxps",
                                           name="cxps")
                psc, ptbs = psc_of[h], ptb_of[h]
                for it in range(8):
                    # start=True clears the whole PSUM bank: only the first
                    # it-slice of each 512-col bank may set it.
                    nc.tensor.matmul(psc[:, it * 128:(it + 1) * 128],
                                     vsb[j][:, h * (DH + 1):(h + 1) * (DH + 1)],
                                     ptbs[it][:, j * 128:(j + 1) * 128],
                                     start=(j == 0 and it % 4 == 0),
                                     stop=(j == 7 and it % 4 == 3),
                                     skip_group_check=True)

            def emit_cxt_evac(h):
                cxt = cxp.tile([80, N], FP16, tag="cxt", name="cxt")
                nc.vector.tensor_copy(cxt[0:DH + 1, :], psc_of.pop(h)[:])
                ctxu = cxp.tile([128, 8 * 80], FP16, tag="ctxu", name="ctxu")
                ctxu3 = ctxu[:, :].rearrange("p (b c) -> p b c", c=80)
                nc.sync.dma_start_transpose(ctxu3[:, :, :], cxt[:, :])
                cxt_of[h] = ctxu

            def emit_norm_item(h, it):
                ctxu = cxt_of[h]
                lr = st.tile([128, 1], FP32, tag="lr", name="lr")
                nc.vector.reciprocal(lr[:], ctxu[:, it * 80 + DH:it * 80 + DH + 1])
                nc.vector.tensor_scalar_mul(
                    ctxall[it][:, h * DH:(h + 1) * DH],
                    ctxu[:, it * 80:it * 80 + DH], lr[:])
                if it == 7:
                    del cxt_of[h]

            emit_qk_block(6)
            emit_qk_block(0)
            for h in range(14):
                if h + 1 <= 5:
                    emit_qk_block(6 + h + 1)
                    emit_qk_block(h + 1)
                if h < 3:
                    for j in range(3 * h, min(3 * h + 3, 8)):
                        emit_v_tile(j)
                for it in range(8):
                    if h < 12:
                        emit_S_item(h, it)
                    if h >= 2:
                        emit_ctx_mm(h - 2, it)
                    if h >= 3:
                        emit_norm_item(h - 3, it)
                if h >= 2:
                    p_of.pop(h - 2, None)
                    ptb_of.pop(h - 2, None)
                    emit_cxt_evac(h - 2)
            # ---- tail: finish head 11, re-transpose ctx, project; the
            # lo-half of the projection only needs i-blocks 0-3, so it starts
            # while the hi-half blocks are still normalizing.
            ctxT = cxp.tile([128, KT * N], FP16, tag="ctxT", name="ctxT")
            ctxT3 = ctxT[:, :].rearrange("p (t i) -> p t i", i=N)
            ps_proj = [None] * KT

            def emit_proj_half(lo, hi):
                for cp in range(KT):
                    if ps_proj[cp] is None:
                        ps_proj[cp] = ps_mm.tile([128, N], FP32, tag="mm",
                                                 name="mm")
                    ps = ps_proj[cp]
                    for t in range(KT):
                        nc.tensor.matmul(ps[:, lo:hi],
                                         wpsb[t][:, cp * 128:(cp + 1) * 128],
                                         ctxT[:, t * N + lo:t * N + hi],
                                         start=(t == 0), stop=(t == KT - 1))

            for it in range(4):
                emit_norm_item(11, it)
                nc.sync.dma_start_transpose(
                    ctxT3[:, :, it * 128:(it + 1) * 128], ctxall[it][:])
            emit_proj_half(0, 512)
            for it in range(4, 8):
                emit_norm_item(11, it)
                nc.sync.dma_start_transpose(
                    ctxT3[:, :, it * 128:(it + 1) * 128], ctxall[it][:])
            emit_proj_half(512, 1024)
            for cp in range(KT):
                ot = oup.tile([128, N], FP32, tag="osb", name="osb")
                nc.vector.tensor_copy(ot[:], ps_proj[cp][:])
                nc.sync.dma_start(out[cp * 128:(cp + 1) * 128, :], ot[:])

    _fix_drain_waits(nc, mybir, bass_rust)
    return nc


def _prep(w_qkv, w_proj):
    r = np.arange(DIM)
    head, d = r // DH, r % DH
    qcols = d * (3 * HEADS) + 0 * HEADS + head
    kcols = d * (3 * HEADS) + 1 * HEADS + head
    vcols = d * (3 * HEADS) + 2 * HEADS + head
    w = np.asarray(w_qkv, np.float32)
    wqk = np.concatenate([w[:, qcols] * np.float32(DH ** 0.5), w[:, kcols]],
                         axis=1).astype(np.float16)
    wv = np.ascontiguousarray(w[:, vcols]).astype(np.float16)
    wp = np.asarray(w_proj, np.float32).astype(np.float16)
    return wqk, wv, wp


def _run(x, w_qkv, w_proj, **spmd_kwargs):
    import sys
    if "/opt/trn_rl_repo" not in sys.path:
        sys.path.insert(0, "/opt/trn_rl_repo")
    from concourse.bass_utils import run_bass_kernel_spmd

    if "nc" not in _cache:
        _cache["nc"] = _build()
    nc = _cache["nc"]

    x = np.asarray(x, np.float32)
    wqk, wv, wp = _prep(w_qkv, w_proj)
    xTs = x.reshape(NB, DIM, N).astype(np.float16)

    in_maps = [
        {"xT": xTs[b], "wqk": wqk, "wv": wv, "wp": wp} for b in range(NB)
    ]
    res = run_bass_kernel_spmd(nc, in_maps, list(range(NB)), **spmd_kwargs)
    outs = np.stack([np.asarray(res.results[b]["out"], np.float32)
                     for b in range(NB)])
    return outs.reshape(NB, DIM, 32, 32), res


def kernel(x, w_qkv, w_proj):
    return _run(x, w_qkv, w_proj)[0]



# revision 18
# speedup vs baseline: 1.0412x; 1.0412x over previous
"""MultiHeadAttention TRN2 kernel v2: batch-parallel across 8 NeuronCores.

Per core (one batch element), vs v1:
  - S matmuls (K=64) packed 2 heads per PE pass via row tile_position 0/64.
  - ctx matmuls (M=64) packed 2 heads via col tile_position 0/64 (no ones
    column in V; row-sums l come free from ACT exp accum_out).
  - P is NOT normalized; 1/l folds into the ctx-PSUM evacuation as a DVE
    tensor_tensor multiply against a per-group broadcast tile R built from
    l via PE-transpose + DVE reciprocal + SWDGE partition-broadcast DMA.
  - P^T via one batched DMA transpose per (group, it): [128, 2048] with a
    4D sliced destination AP writing both heads' per-j-block tiles.

Layout per core:
  xT   [768, 1024] fp16   x[b] as (c, i)
  wqk  [768, 1536] fp16   [q_h0*8 .. q_h11*8, k_h0 .. k_h11] head-major cols
  wv   [768, 768]  fp16   v cols per head (h*64+d)
  wp   [768, 768]  fp16
  out  [768, 1024] fp32   out^T == (c, h, w)
"""
import numpy as np

HEADS, DH, DIM, N = 12, 64, 768, 1024
NB = 8  # batch == cores
KT = DIM // 128  # 6 contraction tiles
HALVES = ((0, 512), (512, 1024))

_cache = {}


def _fix_drain_waits(nc, mybir, bass_rust):
    """Hoist excess per-instruction sync waits onto standalone event-semaphore
    instructions (walrus per-instruction wait budgets)."""
    n = 0
    for f in nc.m.functions:
        for bb in f.blocks:
            new = []
            for ins in bb.instructions:
                si = ins.sync_info
                waits = list(si.on_wait) if si and si.on_wait else []
                limit = 0 if isinstance(ins, mybir.InstDrain) else 1
                if isinstance(ins, mybir.InstEventSemaphore):
                    limit = 2
                if len(waits) > limit:
                    keep, excess = waits[:limit], waits[limit:]
                    for c in range(0, len(excess), 2):
                        n += 1
                        ev = mybir.InstEventSemaphore(
                            name=f"{ins.name}-hoistw{n}", ins=[], outs=[])
                        ev.engine = ins.engine
                        ev.sync_info = bass_rust.SyncInfo(
                            on_wait=excess[c:c + 2], on_update=[])
                        nc.register_instruction(ev, overwrite=True)
                        new.append(ev)
                    si.on_wait = keep
                new.append(ins)
            bb.instructions[:] = new
    return n


def _build():
    import sys
    if "/opt/trn_rl_repo" not in sys.path:
        sys.path.insert(0, "/opt/trn_rl_repo")
    import bass_rust
    import concourse.bass as bass
    import concourse.mybir as mybir
    import concourse.tile as tile
    from concourse.masks import make_identity

    FP16, FP32 = mybir.dt.float16, mybir.dt.float32
    AX = mybir.AxisListType.X
    EXP = mybir.ActivationFunctionType.Exp

    nc = bass.Bass()
    xT = nc.declare_dram_parameter("xT", [DIM, N], FP16, isOutput=False)
    wqk = nc.declare_dram_parameter("wqk", [DIM, 2 * DIM], FP16, isOutput=False)
    wv = nc.declare_dram_parameter("wv", [DIM, DIM], FP16, isOutput=False)
    wp = nc.declare_dram_parameter("wp", [DIM, DIM], FP16, isOutput=False)
    out = nc.declare_dram_parameter("out", [DIM, N], FP32, isOutput=True)

    with tile.TileContext(nc) as tc:
        with (
            tc.tile_pool(name="win", bufs=1) as win,
            tc.tile_pool(name="qk", bufs=1) as qkp,
            tc.tile_pool(name="vp", bufs=1) as vp,
            tc.tile_pool(name="pp", bufs=3) as pp,
            tc.tile_pool(name="pt", bufs=1) as ptp,
            tc.tile_pool(name="st", bufs=8) as st,
            tc.tile_pool(name="lt", bufs=1) as lt,
            tc.tile_pool(name="cx", bufs=1) as cxp,
            tc.tile_pool(name="ou", bufs=2) as oup,
            tc.tile_pool(name="ps_mm", bufs=3, space="PSUM") as ps_mm,
            tc.tile_pool(name="ps_cx", bufs=1, space="PSUM") as ps_cx,
        ):
            # ---- resident loads
            xsb, wqksb, wvsb, wpsb = [], [], [], []
            for t in range(KT):
                xt = win.tile([128, N], FP16, tag=f"x{t}", name=f"x{t}")
                nc.sync.dma_start(xt[:], xT[t * 128:(t + 1) * 128, :])
                xsb.append(xt)
                wt = win.tile([128, 2 * DIM], FP16, tag=f"wqk{t}", name=f"wqk{t}")
                nc.sync.dma_start(wt[:], wqk[t * 128:(t + 1) * 128, :])
                wqksb.append(wt)
            for t in range(KT):
                vt = win.tile([128, DIM], FP16, tag=f"wv{t}", name=f"wv{t}")
                nc.sync.dma_start(vt[:], wv[t * 128:(t + 1) * 128, :])
                wvsb.append(vt)
                pt_ = win.tile([128, DIM], FP16, tag=f"wp{t}", name=f"wp{t}")
                nc.sync.dma_start(pt_[:], wp[t * 128:(t + 1) * 128, :])
                wpsb.append(pt_)
            idt32 = win.tile([128, 128], FP32, tag="idt32", name="idt32")
            make_identity(nc, idt32[:])
            ones1 = win.tile([1, 64], FP16, tag="ones1", name="ones1")
            nc.gpsimd.memset(ones1[:], 1.0)

            # ---- QK^T projection blocks: rows m*128 of [q^T; k^T]
            qksb = [None] * 12

            def emit_qk_block(m):
                ps = ps_mm.tile([128, N], FP32, tag="mm", name="mm")
                for t in range(KT):
                    for lo, hi in HALVES:
                        nc.tensor.matmul(
                            ps[:, lo:hi],
                            wqksb[t][:, m * 128:(m + 1) * 128],
                            xsb[t][:, lo:hi],
                            start=(t == 0), stop=(t == KT - 1),
                        )
                qt = qkp.tile([128, N], FP16, tag=f"qk{m}", name=f"qk{m}")
                if m % 2 == 0:
                    nc.scalar.copy(qt[:], ps[:])
                else:
                    nc.vector.tensor_copy(qt[:], ps[:])
                qksb[m] = qt

            # ---- V projection: V [1024, 768] j-tiles, plain head-major cols
            vsb = [None] * 8

            def emit_v_tile(j):
                ps = ps_mm.tile([128, N], FP32, tag="mm", name="mm")
                for t in range(KT):
                    nc.tensor.matmul(ps[:, 0:512], xsb[t][:, j * 128:(j + 1) * 128],
                                     wvsb[t][:, 0:512],
                                     start=(t == 0), stop=(t == KT - 1))
                    nc.tensor.matmul(ps[:, 512:768], xsb[t][:, j * 128:(j + 1) * 128],
                                     wvsb[t][:, 512:768],
                                     start=(t == 0), stop=(t == KT - 1))
                vt = vp.tile([128, DIM], FP16, tag=f"v{j}", name=f"v{j}")
                nc.scalar.copy(vt[:], ps[:, 0:DIM])
                vsb[j] = vt

            # ---- per-group state (group g = heads 2g, 2g+1)
            PT = {}    # g -> [128, 16384] fp16: [p, (h, b, i)] transposed P
            PT4 = {}
            Lg = {}    # g -> [128, 16] fp32: accum l, col = it*2 + h
            Rg = {}    # g -> [128, 1024] fp16 broadcast 1/l
            psc = {}   # g -> ctx psum [128, 1024]
            ctxT = [None] * 6

            def start_group(g):
                ptg = ptp.tile([128, 16 * N], FP16, tag=f"pt{g % 2}",
                               name=f"pt{g}")
                PT[g] = ptg
                PT4[g] = ptg[:, :].rearrange("p (h b i) -> p h b i", h=2, i=N)
                Lg[g] = lt.tile([128, 16], FP32, tag=f"L{g % 2}", name=f"L{g}")

            def emit_S_step(g, it):
                psA = ps_mm.tile([128, N], FP32, tag="mm", name="mm")
                psB = ps_mm.tile([128, N], FP32, tag="mm", name="mm")
                q, k = qksb[g], qksb[6 + g]
                isl = slice(it * 128, (it + 1) * 128)
                for lo, hi in HALVES:
                    nc.tensor.matmul(psA[:, lo:hi], q[0:64, isl],
                                     k[0:64, lo:hi], start=True, stop=True)
                    nc.tensor.matmul(psB[:, lo:hi], q[64:128, isl],
                                     k[64:128, lo:hi], start=True, stop=True)
                return psA, psB

            def emit_softmax(g, it, psA, psB):
                PPt = pp.tile([128, 2048], FP16, tag="pp", name=f"pp{g}_{it}")
                for h, psx in ((0, psA), (1, psB)):
                    negmax = st.tile([128, 1], FP32, tag="negmax", name="negmax")
                    nc.vector.tensor_reduce(negmax[:], psx[:], axis=AX,
                                            op=mybir.AluOpType.max, negate=True)
                    nc.scalar.activation(
                        PPt[:, h * N:(h + 1) * N], psx[:], EXP,
                        bias=negmax[:], scale=1.0,
                        accum_out=Lg[g][:, h * 8 + it:h * 8 + it + 1])
                return PPt

            def emit_transpose(g, it, PPt):
                nc.sync.dma_start_transpose(
                    PT4[g][:, :, :, it * 128:(it + 1) * 128], PPt[:])

            def emit_R_build(g):
                # l [128 q, 16 (it,h)] -> PE transpose -> [16, 128] -> recip
                # -> SWDGE broadcast to R [128, 1024] (rows 0:64 head A).
                pst = ps_mm.tile([128, N], FP32, tag="mm", name="mm")
                nc.tensor.transpose(pst[0:16, 0:128], Lg[g][:, :], idt32[:])
                rT = lt.tile([16, 128], FP16, tag=f"rT{g % 2}", name=f"rT{g}")
                with nc.allow_low_precision(reason="1/l broadcast tile in fp16"):
                    nc.vector.reciprocal(rT[:], pst[0:16, 0:128])
                # broadcast each rT row across 64 partitions via K=1 matmuls
                rT1 = lt.tile([1, 2048], FP16, tag=f"rT1{g % 2}", name=f"rT1{g}")
                nc.gpsimd.dma_start(
                    rT1[0:1, :].rearrange("p (r c) -> p r c", c=128), rT[:, :])
                psR = ps_mm.tile([128, N], FP32, tag="mm", name="mm")
                for h in range(2):
                    for it in range(8):
                        r = h * 8 + it
                        nc.tensor.matmul(
                            psR[h * 64:(h + 1) * 64, it * 128:(it + 1) * 128],
                            ones1[:, :], rT1[0:1, r * 128:(r + 1) * 128],
                            start=True, stop=True, skip_group_check=True,
                            tile_position=(0, h * 64))
                R = cxp.tile([128, N], FP16, tag=f"R{g % 2}", name=f"R{g}")
                if g % 2 == 0:
                    nc.scalar.copy(R[:], psR[:])
                else:
                    nc.vector.tensor_copy(R[:], psR[:])
                Rg[g] = R

            def emit_ctx_step(g, j):
                if j == 0:
                    psc[g] = ps_cx.tile([128, N], FP32, tag="c", name=f"c{g}")
                pc = psc[g]
                for lo, hi in HALVES:
                    for h in range(2):
                        base = (h * 8 + j) * N
                        nc.tensor.matmul(
                            pc[h * 64:(h + 1) * 64, lo:hi],
                            vsb[j][:, (2 * g + h) * DH:(2 * g + h + 1) * DH],
                            PT[g][:, base + lo:base + hi],
                            start=(j == 0), stop=(j == 7),
                            skip_group_check=True)

            def emit_ctx_evac(g):
                ct = cxp.tile([128, N], FP16, tag=f"cx{g}", name=f"cx{g}")
                nc.vector.tensor_mul(ct[:], psc[g][:], Rg[g][:])
                ctxT[g] = ct
                del psc[g], PT[g], PT4[g], Rg[g]

            def emit_proj(cp):
                ps = ps_mm.tile([128, N], FP32, tag="mm", name="mm")
                for t in range(KT):
                    for lo, hi in HALVES:
                        nc.tensor.matmul(ps[:, lo:hi],
                                         wpsb[t][:, cp * 128:(cp + 1) * 128],
                                         ctxT[t][:, lo:hi],
                                         start=(t == 0), stop=(t == KT - 1))
                ot = oup.tile([128, N], FP32, tag="osb", name="osb")
                nc.scalar.copy(ot[:], ps[:])
                nc.sync.dma_start(out[cp * 128:(cp + 1) * 128, :], ot[:])

            # ---- driver: software-pipelined emission
            emit_qk_block(6)
            emit_qk_block(0)
            filler = ([("qk", 7), ("qk", 1)] + [("v", j) for j in range(8)]
                      + [("qk", 8), ("qk", 2), ("qk", 9), ("qk", 3),
                         ("qk", 10), ("qk", 4), ("qk", 11), ("qk", 5)])
            for g in range(6):
                start_group(g)
                for it in range(8):
                    psA, psB = emit_S_step(g, it)
                    if filler:
                        kind, idx = filler.pop(0)
                        (emit_qk_block if kind == "qk" else emit_v_tile)(idx)
                    if g >= 1:
                        emit_ctx_step(g - 1, it)
                    PPt = emit_softmax(g, it, psA, psB)
                    emit_transpose(g, it, PPt)
                emit_R_build(g)
                if g >= 1:
                    emit_ctx_evac(g - 1)
            for j in range(8):
                emit_ctx_step(5, j)
            emit_ctx_evac(5)
            for cp in range(KT):
                emit_proj(cp)

    _fix_drain_waits(nc, mybir, bass_rust)
    return nc


def _prep(w_qkv, w_proj):
    r = np.arange(DIM)
    head, d = r // DH, r % DH
    qcols = d * (3 * HEADS) + 0 * HEADS + head
    kcols = d * (3 * HEADS) + 1 * HEADS + head
    vcols = d * (3 * HEADS) + 2 * HEADS + head
    w = np.asarray(w_qkv, np.float32)
    wqk = np.concatenate([w[:, qcols] * np.float32(DH ** 0.5), w[:, kcols]],
                         axis=1).astype(np.float16)
    wv = np.ascontiguousarray(w[:, vcols]).astype(np.float16)
    wp = np.asarray(w_proj, np.float32).astype(np.float16)
    return wqk, wv, wp


def _run(x, w_qkv, w_proj, **spmd_kwargs):
    import sys
    if "/opt/trn_rl_repo" not in sys.path:
        sys.path.insert(0, "/opt/trn_rl_repo")
    from concourse.bass_utils import run_bass_kernel_spmd

    if "nc" not in _cache:
        _cache["nc"] = _build()
    nc = _cache["nc"]

    x = np.asarray(x, np.float32)
    wqk, wv, wp = _prep(w_qkv, w_proj)
    xTs = x.reshape(NB, DIM, N).astype(np.float16)

    in_maps = [
        {"xT": xTs[b], "wqk": wqk, "wv": wv, "wp": wp} for b in range(NB)
    ]
    res = run_bass_kernel_spmd(nc, in_maps, list(range(NB)), **spmd_kwargs)
    outs = np.stack([np.asarray(res.results[b]["out"], np.float32)
                     for b in range(NB)])
    return outs.reshape(NB, DIM, 32, 32), res


def kernel(x, w_qkv, w_proj):
    return _run(x, w_qkv, w_proj)[0]


# revision 20
# speedup vs baseline: 1.1868x; 1.1398x over previous
"""MultiHeadAttention TRN2 kernel v2: batch-parallel across 8 NeuronCores.

Per core (one batch element), vs v1:
  - S matmuls (K=64) packed 2 heads per PE pass via row tile_position 0/64.
  - ctx matmuls (M=64) packed 2 heads via col tile_position 0/64 (no ones
    column in V; row-sums l come free from ACT exp accum_out).
  - P is NOT normalized; 1/l folds into the ctx-PSUM evacuation as a DVE
    tensor_tensor multiply against a per-group broadcast tile R built from
    l via PE-transpose + DVE reciprocal + SWDGE partition-broadcast DMA.
  - P^T via one batched DMA transpose per (group, it): [128, 2048] with a
    4D sliced destination AP writing both heads' per-j-block tiles.

Layout per core:
  xT   [768, 1024] fp16   x[b] as (c, i)
  wqk  [768, 1536] fp16   [q_h0*8 .. q_h11*8, k_h0 .. k_h11] head-major cols
  wv   [768, 768]  fp16   v cols per head (h*64+d)
  wp   [768, 768]  fp16
  out  [768, 1024] fp32   out^T == (c, h, w)
"""
import numpy as np

HEADS, DH, DIM, N = 12, 64, 768, 1024
NB = 8  # batch == cores
KT = DIM // 128  # 6 contraction tiles
HALVES = ((0, 512), (512, 1024))

_cache = {}


def _fix_drain_waits(nc, mybir, bass_rust):
    """Hoist excess per-instruction sync waits onto standalone event-semaphore
    instructions (walrus per-instruction wait budgets)."""
    n = 0
    for f in nc.m.functions:
        for bb in f.blocks:
            new = []
            for ins in bb.instructions:
                si = ins.sync_info
                waits = list(si.on_wait) if si and si.on_wait else []
                limit = 0 if isinstance(ins, mybir.InstDrain) else 1
                if isinstance(ins, mybir.InstEventSemaphore):
                    limit = 2
                if len(waits) > limit:
                    keep, excess = waits[:limit], waits[limit:]
                    for c in range(0, len(excess), 2):
                        n += 1
                        ev = mybir.InstEventSemaphore(
                            name=f"{ins.name}-hoistw{n}", ins=[], outs=[])
                        ev.engine = ins.engine
                        ev.sync_info = bass_rust.SyncInfo(
                            on_wait=excess[c:c + 2], on_update=[])
                        nc.register_instruction(ev, overwrite=True)
                        new.append(ev)
                    si.on_wait = keep
                new.append(ins)
            bb.instructions[:] = new
    return n


def _build():
    import sys
    if "/opt/trn_rl_repo" not in sys.path:
        sys.path.insert(0, "/opt/trn_rl_repo")
    import bass_rust
    import concourse.bass as bass
    import concourse.mybir as mybir
    import concourse.tile as tile
    from concourse.masks import make_identity

    FP16, FP32 = mybir.dt.float16, mybir.dt.float32
    AX = mybir.AxisListType.X
    EXP = mybir.ActivationFunctionType.Exp

    nc = bass.Bass()
    xT = nc.declare_dram_parameter("xT", [DIM, N], FP16, isOutput=False)
    wqk = nc.declare_dram_parameter("wqk", [DIM, 2 * DIM], FP16, isOutput=False)
    wv = nc.declare_dram_parameter("wv", [DIM, DIM], FP16, isOutput=False)
    wp = nc.declare_dram_parameter("wp", [DIM, DIM], FP16, isOutput=False)
    out = nc.declare_dram_parameter("out", [DIM, N], FP32, isOutput=True)

    with tile.TileContext(nc) as tc:
        with (
            tc.tile_pool(name="win", bufs=1) as win,
            tc.tile_pool(name="qk", bufs=1) as qkp,
            tc.tile_pool(name="vp", bufs=1) as vp,
            tc.tile_pool(name="pp", bufs=3) as pp,
            tc.tile_pool(name="pt", bufs=1) as ptp,
            tc.tile_pool(name="st", bufs=8) as st,
            tc.tile_pool(name="lt", bufs=1) as lt,
            tc.tile_pool(name="cx", bufs=1) as cxp,
            tc.tile_pool(name="ou", bufs=2) as oup,
            tc.tile_pool(name="ps_mm", bufs=3, space="PSUM") as ps_mm,
            tc.tile_pool(name="ps_cx", bufs=1, space="PSUM") as ps_cx,
        ):
            # ---- resident loads
            xsb, wqksb, wvsb, wpsb = [], [], [], []
            for t in range(KT):
                xt = win.tile([128, N], FP16, tag=f"x{t}", name=f"x{t}")
                nc.sync.dma_start(xt[:], xT[t * 128:(t + 1) * 128, :])
                xsb.append(xt)
                wt = win.tile([128, 2 * DIM], FP16, tag=f"wqk{t}", name=f"wqk{t}")
                nc.sync.dma_start(wt[:], wqk[t * 128:(t + 1) * 128, :])
                wqksb.append(wt)
            for t in range(KT):
                vt = win.tile([128, DIM], FP16, tag=f"wv{t}", name=f"wv{t}")
                nc.sync.dma_start(vt[:], wv[t * 128:(t + 1) * 128, :])
                wvsb.append(vt)
                pt_ = win.tile([128, DIM], FP16, tag=f"wp{t}", name=f"wp{t}")
                nc.sync.dma_start(pt_[:], wp[t * 128:(t + 1) * 128, :])
                wpsb.append(pt_)
            idt32 = win.tile([128, 128], FP32, tag="idt32", name="idt32")
            make_identity(nc, idt32[:])
            ones1 = win.tile([1, 64], FP16, tag="ones1", name="ones1")
            nc.gpsimd.memset(ones1[:], 1.0)

            # ---- QK^T projection blocks: rows m*128 of [q^T; k^T]
            qksb = [None] * 12

            def emit_qk_block(m):
                ps = ps_mm.tile([128, N], FP32, tag="mm", name="mm")
                for t in range(KT):
                    for lo, hi in HALVES:
                        nc.tensor.matmul(
                            ps[:, lo:hi],
                            wqksb[t][:, m * 128:(m + 1) * 128],
                            xsb[t][:, lo:hi],
                            start=(t == 0), stop=(t == KT - 1),
                        )
                qt = qkp.tile([128, N], FP16, tag=f"qk{m}", name=f"qk{m}")
                if m % 2 == 0:
                    nc.scalar.copy(qt[:], ps[:])
                else:
                    nc.vector.tensor_copy(qt[:], ps[:])
                qksb[m] = qt

            # ---- V projection: V [1024, 768] j-tiles, plain head-major cols
            vsb = [None] * 8

            def emit_v_tile(j):
                ps = ps_mm.tile([128, N], FP32, tag="mm", name="mm")
                for t in range(KT):
                    nc.tensor.matmul(ps[:, 0:512], xsb[t][:, j * 128:(j + 1) * 128],
                                     wvsb[t][:, 0:512],
                                     start=(t == 0), stop=(t == KT - 1))
                    nc.tensor.matmul(ps[:, 512:768], xsb[t][:, j * 128:(j + 1) * 128],
                                     wvsb[t][:, 512:768],
                                     start=(t == 0), stop=(t == KT - 1))
                vt = vp.tile([128, DIM], FP16, tag=f"v{j}", name=f"v{j}")
                nc.scalar.copy(vt[:], ps[:, 0:DIM])
                vsb[j] = vt

            # ---- per-group state (group g = heads 2g, 2g+1)
            PT = {}    # g -> [128, 16384] fp16: [p, (h, b, i)] transposed P
            PT4 = {}
            Lg = {}    # g -> [128, 16] fp32: accum l, col = it*2 + h
            Rg = {}    # g -> [128, 1024] fp16 broadcast 1/l
            psc = {}   # g -> ctx psum [128, 1024]
            ctxT = [None] * 6

            def start_group(g):
                ptg = ptp.tile([128, 16 * N], FP16, tag=f"pt{g % 2}",
                               name=f"pt{g}")
                PT[g] = ptg
                PT4[g] = ptg[:, :].rearrange("p (h b i) -> p h b i", h=2, i=N)
                Lg[g] = lt.tile([128, 16], FP32, tag=f"L{g % 2}", name=f"L{g}")

            def emit_S_step(g, it):
                psA = ps_mm.tile([128, N], FP32, tag="mm", name="mm")
                psB = ps_mm.tile([128, N], FP32, tag="mm", name="mm")
                q, k = qksb[g], qksb[6 + g]
                isl = slice(it * 128, (it + 1) * 128)
                for lo, hi in HALVES:
                    nc.tensor.matmul(psA[:, lo:hi], q[0:64, isl],
                                     k[0:64, lo:hi], start=True, stop=True)
                    nc.tensor.matmul(psB[:, lo:hi], q[64:128, isl],
                                     k[64:128, lo:hi], start=True, stop=True)
                return psA, psB

            def emit_softmax(g, it, psA, psB):
                PPt = pp.tile([128, 2048], FP16, tag="pp", name=f"pp{g}_{it}")
                for h, psx in ((0, psA), (1, psB)):
                    negmax = st.tile([128, 1], FP32, tag="negmax", name="negmax")
                    nc.vector.tensor_reduce(negmax[:], psx[:], axis=AX,
                                            op=mybir.AluOpType.max, negate=True)
                    nc.scalar.activation(
                        PPt[:, h * N:(h + 1) * N], psx[:], EXP,
                        bias=negmax[:], scale=1.0,
                        accum_out=Lg[g][:, h * 8 + it:h * 8 + it + 1])
                return PPt

            def emit_transpose(g, it, PPt):
                nc.sync.dma_start_transpose(
                    PT4[g][:, :, :, it * 128:(it + 1) * 128], PPt[:])

            rT1g = {}

            def emit_R_build_a(g):
                # l [128 q, 16 (h,it)] -> PE transpose -> [16, 128] -> recip
                # -> SWDGE reshape to one partition [1, 2048].
                pst = ps_mm.tile([128, N], FP32, tag="mm", name="mm")
                nc.tensor.transpose(pst[0:16, 0:128], Lg[g][:, :], idt32[:])
                rT = lt.tile([16, 128], FP16, tag=f"rT{g % 2}", name=f"rT{g}")
                with nc.allow_low_precision(reason="1/l broadcast tile in fp16"):
                    nc.vector.reciprocal(rT[:], pst[0:16, 0:128])
                rT1 = lt.tile([1, 2048], FP16, tag=f"rT1{g % 2}", name=f"rT1{g}")
                nc.gpsimd.dma_start(
                    rT1[0:1, :].rearrange("p (r c) -> p r c", c=128), rT[:, :])
                rT1g[g] = rT1

            def emit_R_build_b(g):
                # broadcast each 1/l row across 64 partitions via K=1 matmuls
                rT1 = rT1g.pop(g)
                psR = ps_mm.tile([128, N], FP32, tag="mm", name="mm")
                for h in range(2):
                    for it in range(8):
                        r = h * 8 + it
                        nc.tensor.matmul(
                            psR[h * 64:(h + 1) * 64, it * 128:(it + 1) * 128],
                            ones1[:, :], rT1[0:1, r * 128:(r + 1) * 128],
                            start=True, stop=True, skip_group_check=True,
                            tile_position=(0, h * 64))
                R = cxp.tile([128, N], FP16, tag=f"R{g % 2}", name=f"R{g}")
                if g % 2 == 0:
                    nc.scalar.copy(R[:], psR[:])
                else:
                    nc.vector.tensor_copy(R[:], psR[:])
                Rg[g] = R

            def emit_ctx_step(g, j):
                if j == 0:
                    psc[g] = ps_cx.tile([128, N], FP32, tag="c", name=f"c{g}")
                pc = psc[g]
                for lo, hi in HALVES:
                    for h in range(2):
                        base = (h * 8 + j) * N
                        nc.tensor.matmul(
                            pc[h * 64:(h + 1) * 64, lo:hi],
                            vsb[j][:, (2 * g + h) * DH:(2 * g + h + 1) * DH],
                            PT[g][:, base + lo:base + hi],
                            start=(j == 0), stop=(j == 7),
                            skip_group_check=True)

            def emit_ctx_evac(g):
                ct = cxp.tile([128, N], FP16, tag=f"cx{g}", name=f"cx{g}")
                nc.vector.tensor_mul(ct[:], psc[g][:], Rg[g][:])
                ctxT[g] = ct
                del psc[g], PT[g], PT4[g], Rg[g]

            def emit_proj(cp):
                ps = ps_mm.tile([128, N], FP32, tag="mm", name="mm")
                for t in range(KT):
                    for lo, hi in HALVES:
                        nc.tensor.matmul(ps[:, lo:hi],
                                         wpsb[t][:, cp * 128:(cp + 1) * 128],
                                         ctxT[t][:, lo:hi],
                                         start=(t == 0), stop=(t == KT - 1))
                ot = oup.tile([128, N], FP32, tag="osb", name="osb")
                nc.scalar.copy(ot[:], ps[:])
                nc.sync.dma_start(out[cp * 128:(cp + 1) * 128, :], ot[:])

            # ---- driver: software-pipelined emission
            emit_qk_block(6)
            emit_qk_block(0)
            filler = ([("qk", 7), ("qk", 1)] + [("v", j) for j in range(8)]
                      + [("qk", 8), ("qk", 2), ("qk", 9), ("qk", 3),
                         ("qk", 10), ("qk", 4), ("qk", 11), ("qk", 5)])
            for g in range(6):
                start_group(g)
                for it in range(8):
                    psA, psB = emit_S_step(g, it)
                    if filler:
                        kind, idx = filler.pop(0)
                        (emit_qk_block if kind == "qk" else emit_v_tile)(idx)
                    if g >= 1:
                        emit_ctx_step(g - 1, it)
                        if it == 1:
                            emit_R_build_a(g - 1)
                        elif it == 4:
                            emit_R_build_b(g - 1)
                    PPt = emit_softmax(g, it, psA, psB)
                    emit_transpose(g, it, PPt)
                if g >= 1:
                    emit_ctx_evac(g - 1)
            for j in range(8):
                emit_ctx_step(5, j)
            emit_R_build_a(5)
            emit_R_build_b(5)
            emit_ctx_evac(5)
            for cp in range(KT):
                emit_proj(cp)

    _fix_drain_waits(nc, mybir, bass_rust)
    return nc


def _prep(w_qkv, w_proj):
    r = np.arange(DIM)
    head, d = r // DH, r % DH
    qcols = d * (3 * HEADS) + 0 * HEADS + head
    kcols = d * (3 * HEADS) + 1 * HEADS + head
    vcols = d * (3 * HEADS) + 2 * HEADS + head
    w = np.asarray(w_qkv, np.float32)
    wqk = np.concatenate([w[:, qcols] * np.float32(DH ** 0.5), w[:, kcols]],
                         axis=1).astype(np.float16)
    wv = np.ascontiguousarray(w[:, vcols]).astype(np.float16)
    wp = np.asarray(w_proj, np.float32).astype(np.float16)
    return wqk, wv, wp


def _run(x, w_qkv, w_proj, **spmd_kwargs):
    import sys
    if "/opt/trn_rl_repo" not in sys.path:
        sys.path.insert(0, "/opt/trn_rl_repo")
    from concourse.bass_utils import run_bass_kernel_spmd

    if "nc" not in _cache:
        _cache["nc"] = _build()
    nc = _cache["nc"]

    x = np.asarray(x, np.float32)
    wqk, wv, wp = _prep(w_qkv, w_proj)
    xTs = x.reshape(NB, DIM, N).astype(np.float16)

    in_maps = [
        {"xT": xTs[b], "wqk": wqk, "wv": wv, "wp": wp} for b in range(NB)
    ]
    res = run_bass_kernel_spmd(nc, in_maps, list(range(NB)), **spmd_kwargs)
    outs = np.stack([np.asarray(res.results[b]["out"], np.float32)
                     for b in range(NB)])
    return outs.reshape(NB, DIM, 32, 32), res


def kernel(x, w_qkv, w_proj):
    return _run(x, w_qkv, w_proj)[0]


# revision 21
# speedup vs baseline: 1.1897x; 1.0024x over previous
"""MultiHeadAttention TRN2 kernel v2: batch-parallel across 8 NeuronCores.

Per core (one batch element), vs v1:
  - S matmuls (K=64) packed 2 heads per PE pass via row tile_position 0/64.
  - ctx matmuls (M=64) packed 2 heads via col tile_position 0/64 (no ones
    column in V; row-sums l come free from ACT exp accum_out).
  - P is NOT normalized; 1/l folds into the ctx-PSUM evacuation as a DVE
    tensor_tensor multiply against a per-group broadcast tile R built from
    l via PE-transpose + DVE reciprocal + SWDGE partition-broadcast DMA.
  - P^T via one batched DMA transpose per (group, it): [128, 2048] with a
    4D sliced destination AP writing both heads' per-j-block tiles.

Layout per core:
  xT   [768, 1024] fp16   x[b] as (c, i)
  wqk  [768, 1536] fp16   [q_h0*8 .. q_h11*8, k_h0 .. k_h11] head-major cols
  wv   [768, 768]  fp16   v cols per head (h*64+d)
  wp   [768, 768]  fp16
  out  [768, 1024] fp32   out^T == (c, h, w)
"""
import numpy as np

HEADS, DH, DIM, N = 12, 64, 768, 1024
NB = 8  # batch == cores
KT = DIM // 128  # 6 contraction tiles
HALVES = ((0, 512), (512, 1024))

_cache = {}


def _fix_drain_waits(nc, mybir, bass_rust):
    """Hoist excess per-instruction sync waits onto standalone event-semaphore
    instructions (walrus per-instruction wait budgets)."""
    n = 0
    for f in nc.m.functions:
        for bb in f.blocks:
            new = []
            for ins in bb.instructions:
                si = ins.sync_info
                waits = list(si.on_wait) if si and si.on_wait else []
                limit = 0 if isinstance(ins, mybir.InstDrain) else 1
                if isinstance(ins, mybir.InstEventSemaphore):
                    limit = 2
                if len(waits) > limit:
                    keep, excess = waits[:limit], waits[limit:]
                    for c in range(0, len(excess), 2):
                        n += 1
                        ev = mybir.InstEventSemaphore(
                            name=f"{ins.name}-hoistw{n}", ins=[], outs=[])
                        ev.engine = ins.engine
                        ev.sync_info = bass_rust.SyncInfo(
                            on_wait=excess[c:c + 2], on_update=[])
                        nc.register_instruction(ev, overwrite=True)
                        new.append(ev)
                    si.on_wait = keep
                new.append(ins)
            bb.instructions[:] = new
    return n


def _build():
    import sys
    if "/opt/trn_rl_repo" not in sys.path:
        sys.path.insert(0, "/opt/trn_rl_repo")
    import bass_rust
    import concourse.bass as bass
    import concourse.mybir as mybir
    import concourse.tile as tile
    from concourse.masks import make_identity

    FP16, FP32 = mybir.dt.float16, mybir.dt.float32
    AX = mybir.AxisListType.X
    EXP = mybir.ActivationFunctionType.Exp

    nc = bass.Bass()
    xT = nc.declare_dram_parameter("xT", [DIM, N], FP16, isOutput=False)
    wqk = nc.declare_dram_parameter("wqk", [DIM, 2 * DIM], FP16, isOutput=False)
    wv = nc.declare_dram_parameter("wv", [DIM, DIM], FP16, isOutput=False)
    wp = nc.declare_dram_parameter("wp", [DIM, DIM], FP16, isOutput=False)
    out = nc.declare_dram_parameter("out", [DIM, N], FP32, isOutput=True)

    with tile.TileContext(nc) as tc:
        with (
            tc.tile_pool(name="win", bufs=1) as win,
            tc.tile_pool(name="qk", bufs=1) as qkp,
            tc.tile_pool(name="vp", bufs=1) as vp,
            tc.tile_pool(name="pp", bufs=3) as pp,
            tc.tile_pool(name="pt", bufs=1) as ptp,
            tc.tile_pool(name="st", bufs=8) as st,
            tc.tile_pool(name="lt", bufs=1) as lt,
            tc.tile_pool(name="cx", bufs=1) as cxp,
            tc.tile_pool(name="ou", bufs=2) as oup,
            tc.tile_pool(name="ps_mm", bufs=3, space="PSUM") as ps_mm,
            tc.tile_pool(name="ps_cx", bufs=1, space="PSUM") as ps_cx,
        ):
            # ---- resident loads
            xsb, wqksb, wvsb, wpsb = [], [], [], []
            for t in range(KT):
                xt = win.tile([128, N], FP16, tag=f"x{t}", name=f"x{t}")
                nc.sync.dma_start(xt[:], xT[t * 128:(t + 1) * 128, :])
                xsb.append(xt)
                wt = win.tile([128, 2 * DIM], FP16, tag=f"wqk{t}", name=f"wqk{t}")
                nc.sync.dma_start(wt[:], wqk[t * 128:(t + 1) * 128, :])
                wqksb.append(wt)
            for t in range(KT):
                vt = win.tile([128, DIM], FP16, tag=f"wv{t}", name=f"wv{t}")
                nc.sync.dma_start(vt[:], wv[t * 128:(t + 1) * 128, :])
                wvsb.append(vt)
                pt_ = win.tile([128, DIM], FP16, tag=f"wp{t}", name=f"wp{t}")
                nc.sync.dma_start(pt_[:], wp[t * 128:(t + 1) * 128, :])
                wpsb.append(pt_)
            idt32 = win.tile([128, 128], FP32, tag="idt32", name="idt32")
            make_identity(nc, idt32[:])
            ones1 = win.tile([1, 64], FP16, tag="ones1", name="ones1")
            nc.gpsimd.memset(ones1[:], 1.0)

            # ---- QK^T projection blocks: rows m*128 of [q^T; k^T]
            qksb = [None] * 12

            def emit_qk_block(m):
                ps = ps_mm.tile([128, N], FP32, tag="mm", name="mm")
                for t in range(KT):
                    for lo, hi in HALVES:
                        nc.tensor.matmul(
                            ps[:, lo:hi],
                            wqksb[t][:, m * 128:(m + 1) * 128],
                            xsb[t][:, lo:hi],
                            start=(t == 0), stop=(t == KT - 1),
                        )
                qt = qkp.tile([128, N], FP16, tag=f"qk{m}", name=f"qk{m}")
                if m % 2 == 0:
                    nc.scalar.copy(qt[:], ps[:])
                else:
                    nc.vector.tensor_copy(qt[:], ps[:])
                qksb[m] = qt

            # ---- V projection: V [1024, 768] j-tiles, plain head-major cols
            vsb = [None] * 8

            def emit_v_tile(j):
                ps = ps_mm.tile([128, N], FP32, tag="mm", name="mm")
                for t in range(KT):
                    nc.tensor.matmul(ps[:, 0:512], xsb[t][:, j * 128:(j + 1) * 128],
                                     wvsb[t][:, 0:512],
                                     start=(t == 0), stop=(t == KT - 1))
                    nc.tensor.matmul(ps[:, 512:768], xsb[t][:, j * 128:(j + 1) * 128],
                                     wvsb[t][:, 512:768],
                                     start=(t == 0), stop=(t == KT - 1))
                vt = vp.tile([128, DIM], FP16, tag=f"v{j}", name=f"v{j}")
                nc.scalar.copy(vt[:], ps[:, 0:DIM])
                vsb[j] = vt

            # ---- per-group state (group g = heads 2g, 2g+1)
            PT = {}    # g -> [128, 16384] fp16: [p, (h, b, i)] transposed P
            PT4 = {}
            Lg = {}    # g -> [128, 16] fp32: accum l, col = it*2 + h
            Rg = {}    # g -> [128, 1024] fp16 broadcast 1/l
            psc = {}   # g -> ctx psum [128, 1024]
            ctxT = [None] * 6

            def start_group(g):
                ptg = ptp.tile([128, 16 * N], FP16, tag=f"pt{g % 2}",
                               name=f"pt{g}")
                PT[g] = ptg
                PT4[g] = ptg[:, :].rearrange("p (h b i) -> p h b i", h=2, i=N)
                Lg[g] = lt.tile([128, 16], FP32, tag=f"L{g % 2}", name=f"L{g}")

            def emit_S_step(g, it):
                psA = ps_mm.tile([128, N], FP32, tag="mm", name="mm")
                psB = ps_mm.tile([128, N], FP32, tag="mm", name="mm")
                q, k = qksb[g], qksb[6 + g]
                isl = slice(it * 128, (it + 1) * 128)
                for lo, hi in HALVES:
                    nc.tensor.matmul(psA[:, lo:hi], q[0:64, isl],
                                     k[0:64, lo:hi], start=True, stop=True)
                    nc.tensor.matmul(psB[:, lo:hi], q[64:128, isl],
                                     k[64:128, lo:hi], start=True, stop=True)
                return psA, psB

            def emit_softmax(g, it, psA, psB):
                PPt = pp.tile([128, 2048], FP16, tag="pp", name=f"pp{g}_{it}")
                for h, psx in ((0, psA), (1, psB)):
                    negmax = st.tile([128, 1], FP32, tag="negmax", name="negmax")
                    nc.vector.tensor_reduce(negmax[:], psx[:], axis=AX,
                                            op=mybir.AluOpType.max, negate=True)
                    nc.scalar.activation(
                        PPt[:, h * N:(h + 1) * N], psx[:], EXP,
                        bias=negmax[:], scale=1.0,
                        accum_out=Lg[g][:, h * 8 + it:h * 8 + it + 1])
                return PPt

            def emit_transpose(g, it, PPt):
                nc.sync.dma_start_transpose(
                    PT4[g][:, :, :, it * 128:(it + 1) * 128], PPt[:])

            rT1g = {}

            def emit_R_build_a(g):
                # l [128 q, 16 (h,it)] -> PE transpose -> [16, 128] -> recip
                # -> SWDGE reshape to one partition [1, 2048].
                pst = ps_mm.tile([128, N], FP32, tag="mm", name="mm")
                nc.tensor.transpose(pst[0:16, 0:128], Lg[g][:, :], idt32[:])
                rT = lt.tile([16, 128], FP16, tag=f"rT{g % 2}", name=f"rT{g}")
                with nc.allow_low_precision(reason="1/l broadcast tile in fp16"):
                    nc.vector.reciprocal(rT[:], pst[0:16, 0:128])
                rT1 = lt.tile([1, 2048], FP16, tag=f"rT1{g % 2}", name=f"rT1{g}")
                nc.gpsimd.dma_start(
                    rT1[0:1, :].rearrange("p (r c) -> p r c", c=128), rT[:, :])
                rT1g[g] = rT1

            def emit_R_build_b(g):
                # broadcast each 1/l row across 64 partitions via K=1 matmuls
                rT1 = rT1g.pop(g)
                psR = ps_mm.tile([128, N], FP32, tag="mm", name="mm")
                for h in range(2):
                    for it in range(8):
                        r = h * 8 + it
                        nc.tensor.matmul(
                            psR[h * 64:(h + 1) * 64, it * 128:(it + 1) * 128],
                            ones1[:, :], rT1[0:1, r * 128:(r + 1) * 128],
                            start=True, stop=True, skip_group_check=True,
                            tile_position=(0, h * 64))
                R = cxp.tile([128, N], FP16, tag=f"R{g % 2}", name=f"R{g}")
                if g % 2 == 0:
                    nc.scalar.copy(R[:], psR[:])
                else:
                    nc.vector.tensor_copy(R[:], psR[:])
                Rg[g] = R

            def emit_ctx_step(g, j):
                if j == 0:
                    psc[g] = ps_cx.tile([128, N], FP32, tag="c", name=f"c{g}")
                pc = psc[g]
                for lo, hi in HALVES:
                    for h in range(2):
                        base = (h * 8 + j) * N
                        nc.tensor.matmul(
                            pc[h * 64:(h + 1) * 64, lo:hi],
                            vsb[j][:, (2 * g + h) * DH:(2 * g + h + 1) * DH],
                            PT[g][:, base + lo:base + hi],
                            start=(j == 0), stop=(j == 7),
                            skip_group_check=True)

            def emit_ctx_evac(g):
                ct = cxp.tile([128, N], FP16, tag=f"cx{g}", name=f"cx{g}")
                nc.vector.tensor_mul(ct[:], psc[g][:], Rg[g][:])
                ctxT[g] = ct
                del psc[g], PT[g], PT4[g], Rg[g]

            def emit_proj(cp):
                ps = ps_mm.tile([128, N], FP32, tag="mm", name="mm")
                for t in range(KT):
                    for lo, hi in HALVES:
                        nc.tensor.matmul(ps[:, lo:hi],
                                         wpsb[t][:, cp * 128:(cp + 1) * 128],
                                         ctxT[t][:, lo:hi],
                                         start=(t == 0), stop=(t == KT - 1))
                ot = oup.tile([128, N], FP32, tag="osb", name="osb")
                nc.scalar.copy(ot[:], ps[:])
                nc.sync.dma_start(out[cp * 128:(cp + 1) * 128, :], ot[:])

            # ---- driver: software-pipelined emission
            emit_qk_block(6)
            emit_qk_block(0)
            filler = ([("qk", 7), ("qk", 1)] + [("v", j) for j in range(8)]
                      + [("qk", 8), ("qk", 2), ("qk", 9), ("qk", 3),
                         ("qk", 10), ("qk", 4), ("qk", 11), ("qk", 5)])
            for g in range(6):
                start_group(g)
                for it in range(8):
                    psA, psB = emit_S_step(g, it)
                    if filler:
                        kind, idx = filler.pop(0)
                        (emit_qk_block if kind == "qk" else emit_v_tile)(idx)
                    if g >= 1:
                        if it >= 1:
                            emit_ctx_step(g - 1, it - 1)
                        if it == 2:
                            emit_R_build_a(g - 1)
                        elif it == 5:
                            emit_R_build_b(g - 1)
                    PPt = emit_softmax(g, it, psA, psB)
                    emit_transpose(g, it, PPt)
                if g >= 1:
                    emit_ctx_step(g - 1, 7)
                    emit_ctx_evac(g - 1)
            for j in range(8):
                emit_ctx_step(5, j)
            emit_R_build_a(5)
            emit_R_build_b(5)
            emit_ctx_evac(5)
            for cp in range(KT):
                emit_proj(cp)

    _fix_drain_waits(nc, mybir, bass_rust)
    return nc


def _prep(w_qkv, w_proj):
    r = np.arange(DIM)
    head, d = r // DH, r % DH
    qcols = d * (3 * HEADS) + 0 * HEADS + head
    kcols = d * (3 * HEADS) + 1 * HEADS + head
    vcols = d * (3 * HEADS) + 2 * HEADS + head
    w = np.asarray(w_qkv, np.float32)
    wqk = np.concatenate([w[:, qcols] * np.float32(DH ** 0.5), w[:, kcols]],
                         axis=1).astype(np.float16)
    wv = np.ascontiguousarray(w[:, vcols]).astype(np.float16)
    wp = np.asarray(w_proj, np.float32).astype(np.float16)
    return wqk, wv, wp


def _run(x, w_qkv, w_proj, **spmd_kwargs):
    import sys
    if "/opt/trn_rl_repo" not in sys.path:
        sys.path.insert(0, "/opt/trn_rl_repo")
    from concourse.bass_utils import run_bass_kernel_spmd

    if "nc" not in _cache:
        _cache["nc"] = _build()
    nc = _cache["nc"]

    x = np.asarray(x, np.float32)
    wqk, wv, wp = _prep(w_qkv, w_proj)
    xTs = x.reshape(NB, DIM, N).astype(np.float16)

    in_maps = [
        {"xT": xTs[b], "wqk": wqk, "wv": wv, "wp": wp} for b in range(NB)
    ]
    res = run_bass_kernel_spmd(nc, in_maps, list(range(NB)), **spmd_kwargs)
    outs = np.stack([np.asarray(res.results[b]["out"], np.float32)
                     for b in range(NB)])
    return outs.reshape(NB, DIM, 32, 32), res


def kernel(x, w_qkv, w_proj):
    return _run(x, w_qkv, w_proj)[0]


# revision 26
# speedup vs baseline: 1.2302x; 1.0341x over previous
"""MultiHeadAttention TRN2 kernel v2: batch-parallel across 8 NeuronCores.

Per core (one batch element), vs v1:
  - S matmuls (K=64) packed 2 heads per PE pass via row tile_position 0/64.
  - ctx matmuls (M=64) packed 2 heads via col tile_position 0/64 (no ones
    column in V; row-sums l come free from ACT exp accum_out).
  - P is NOT normalized; 1/l folds into the ctx-PSUM evacuation as a DVE
    tensor_tensor multiply against a per-group broadcast tile R built from
    l via PE-transpose + DVE reciprocal + SWDGE partition-broadcast DMA.
  - P^T via one batched DMA transpose per (group, it): [128, 2048] with a
    4D sliced destination AP writing both heads' per-j-block tiles.

Layout per core:
  xT   [768, 1024] fp16   x[b] as (c, i)
  wqk  [768, 1536] fp16   [q_h0*8 .. q_h11*8, k_h0 .. k_h11] head-major cols
  wv   [768, 768]  fp16   v cols per head (h*64+d)
  wp   [768, 768]  fp16
  out  [768, 1024] fp32   out^T == (c, h, w)
"""
import numpy as np

HEADS, DH, DIM, N = 12, 64, 768, 1024
NB = 8  # batch == cores
KT = DIM // 128  # 6 contraction tiles
HALVES = ((0, 512), (512, 1024))

_cache = {}


def _fix_drain_waits(nc, mybir, bass_rust):
    """Hoist excess per-instruction sync waits onto standalone event-semaphore
    instructions (walrus per-instruction wait budgets)."""
    n = 0
    for f in nc.m.functions:
        for bb in f.blocks:
            new = []
            for ins in bb.instructions:
                si = ins.sync_info
                waits = list(si.on_wait) if si and si.on_wait else []
                limit = 0 if isinstance(ins, mybir.InstDrain) else 1
                if isinstance(ins, mybir.InstEventSemaphore):
                    limit = 2
                if len(waits) > limit:
                    keep, excess = waits[:limit], waits[limit:]
                    for c in range(0, len(excess), 2):
                        n += 1
                        ev = mybir.InstEventSemaphore(
                            name=f"{ins.name}-hoistw{n}", ins=[], outs=[])
                        ev.engine = ins.engine
                        ev.sync_info = bass_rust.SyncInfo(
                            on_wait=excess[c:c + 2], on_update=[])
                        nc.register_instruction(ev, overwrite=True)
                        new.append(ev)
                    si.on_wait = keep
                new.append(ins)
            bb.instructions[:] = new
    return n


def _build():
    import sys
    if "/opt/trn_rl_repo" not in sys.path:
        sys.path.insert(0, "/opt/trn_rl_repo")
    import bass_rust
    import concourse.bass as bass
    import concourse.mybir as mybir
    import concourse.tile as tile
    from concourse.masks import make_identity

    FP16, FP32 = mybir.dt.float16, mybir.dt.float32
    AX = mybir.AxisListType.X
    EXP = mybir.ActivationFunctionType.Exp

    nc = bass.Bass()
    xT = nc.declare_dram_parameter("xT", [DIM, N], FP16, isOutput=False)
    wqk = nc.declare_dram_parameter("wqk", [DIM, 2 * DIM], FP16, isOutput=False)
    wv = nc.declare_dram_parameter("wv", [DIM, DIM], FP16, isOutput=False)
    wp = nc.declare_dram_parameter("wp", [DIM, DIM], FP16, isOutput=False)
    out = nc.declare_dram_parameter("out", [DIM, N], FP32, isOutput=True)

    with tile.TileContext(nc) as tc:
        with (
            tc.tile_pool(name="win", bufs=1) as win,
            tc.tile_pool(name="qk", bufs=1) as qkp,
            tc.tile_pool(name="vp", bufs=1) as vp,
            tc.tile_pool(name="pp", bufs=4) as pp,
            tc.tile_pool(name="pt", bufs=1) as ptp,
            tc.tile_pool(name="st", bufs=8) as st,
            tc.tile_pool(name="lt", bufs=1) as lt,
            tc.tile_pool(name="cx", bufs=1) as cxp,
            tc.tile_pool(name="ou", bufs=2) as oup,
            tc.tile_pool(name="ps_mm", bufs=4, space="PSUM") as ps_mm,
        ):
            # ---- resident loads
            xsb, wqksb, wvsb, wpsb = [], [], [], []
            for t in range(KT):
                xt = win.tile([128, N], FP16, tag=f"x{t}", name=f"x{t}")
                nc.sync.dma_start(xt[:], xT[t * 128:(t + 1) * 128, :])
                xsb.append(xt)
                wt = win.tile([128, 2 * DIM], FP16, tag=f"wqk{t}", name=f"wqk{t}")
                nc.gpsimd.dma_start(wt[:], wqk[t * 128:(t + 1) * 128, :])
                wqksb.append(wt)
            for t in range(KT):
                vt = win.tile([128, DIM], FP16, tag=f"wv{t}", name=f"wv{t}")
                nc.gpsimd.dma_start(vt[:], wv[t * 128:(t + 1) * 128, :])
                wvsb.append(vt)
                pt_ = win.tile([128, DIM], FP16, tag=f"wp{t}", name=f"wp{t}")
                nc.gpsimd.dma_start(pt_[:], wp[t * 128:(t + 1) * 128, :])
                wpsb.append(pt_)
            idt32 = win.tile([128, 128], FP32, tag="idt32", name="idt32")
            make_identity(nc, idt32[:])
            ones1 = win.tile([1, 64], FP16, tag="ones1", name="ones1")
            nc.gpsimd.memset(ones1[:], 1.0)

            # ---- QK^T projection blocks: rows m*128 of [q^T; k^T]
            qksb = [None] * 12

            def emit_qk_block(m):
                ps = ps_mm.tile([128, N], FP32, tag="mm", name="mm")
                for t in range(KT):
                    for lo, hi in HALVES:
                        nc.tensor.matmul(
                            ps[:, lo:hi],
                            wqksb[t][:, m * 128:(m + 1) * 128],
                            xsb[t][:, lo:hi],
                            start=(t == 0), stop=(t == KT - 1),
                        )
                qt = qkp.tile([128, N], FP16, tag=f"qk{m}", name=f"qk{m}")
                if m % 2 == 0:
                    nc.scalar.copy(qt[:], ps[:])
                else:
                    nc.vector.tensor_copy(qt[:], ps[:])
                qksb[m] = qt

            # ---- V projection: V [1024, 768] j-tiles, plain head-major cols
            vsb = [None] * 8

            def emit_v_tile(j):
                ps = ps_mm.tile([128, N], FP32, tag="mm", name="mm")
                for t in range(KT):
                    nc.tensor.matmul(ps[:, 0:512], xsb[t][:, j * 128:(j + 1) * 128],
                                     wvsb[t][:, 0:512],
                                     start=(t == 0), stop=(t == KT - 1))
                    nc.tensor.matmul(ps[:, 512:768], xsb[t][:, j * 128:(j + 1) * 128],
                                     wvsb[t][:, 512:768],
                                     start=(t == 0), stop=(t == KT - 1))
                vt = vp.tile([128, DIM], FP16, tag=f"v{j}", name=f"v{j}")
                nc.scalar.copy(vt[:], ps[:, 0:DIM])
                vsb[j] = vt

            # ---- per-group state (group g = heads 2g, 2g+1)
            PT = {}    # g -> [128, 16384] fp16: [p, (h, b, i)] transposed P
            PT4 = {}
            Lg = {}    # g -> [128, 16] fp32: accum l, col = it*2 + h
            Rg = {}    # g -> [128, 1024] fp16 broadcast 1/l
            psc = {}   # g -> ctx psum [128, 1024]
            ctxT = [None] * 6

            def start_group(g):
                ptg = ptp.tile([128, 16 * N], FP16, tag=f"pt{g % 2}",
                               name=f"pt{g}")
                PT[g] = ptg
                PT4[g] = ptg[:, :].rearrange("p (h b i) -> p h b i", h=2, i=N)
                Lg[g] = lt.tile([128, 16], FP32, tag=f"L{g % 2}", name=f"L{g}")

            def emit_S_step(g, it):
                psA = ps_mm.tile([128, N], FP32, tag="mm", name="mm")
                psB = ps_mm.tile([128, N], FP32, tag="mm", name="mm")
                q, k = qksb[g], qksb[6 + g]
                isl = slice(it * 128, (it + 1) * 128)
                for lo, hi in HALVES:
                    nc.tensor.matmul(psA[:, lo:hi], q[0:64, isl],
                                     k[0:64, lo:hi], start=True, stop=True)
                    nc.tensor.matmul(psB[:, lo:hi], q[64:128, isl],
                                     k[64:128, lo:hi], start=True, stop=True)
                return psA, psB

            def emit_softmax(g, it, psA, psB):
                PPt = pp.tile([128, 2048], FP16, tag="pp", name=f"pp{g}_{it}")
                for h, psx in ((0, psA), (1, psB)):
                    negmax = st.tile([128, 1], FP32, tag="negmax", name="negmax")
                    nc.vector.tensor_reduce(negmax[:], psx[:], axis=AX,
                                            op=mybir.AluOpType.max, negate=True)
                    nc.scalar.activation(
                        PPt[:, h * N:(h + 1) * N], psx[:], EXP,
                        bias=negmax[:], scale=1.0,
                        accum_out=Lg[g][:, h * 8 + it:h * 8 + it + 1])
                return PPt

            def emit_transpose(g, it, PPt):
                nc.sync.dma_start_transpose(
                    PT4[g][:, :, :, it * 128:(it + 1) * 128], PPt[:])

            rT1g = {}

            def emit_R_build_a(g):
                # l [128 q, 16 (h,it)] -> PE transpose -> [16, 128] -> recip
                # -> SWDGE reshape to one partition [1, 2048].
                pst = ps_mm.tile([128, N], FP32, tag="mm", name="mm")
                nc.tensor.transpose(pst[0:16, 0:128], Lg[g][:, :], idt32[:])
                rT = lt.tile([16, 128], FP16, tag=f"rT{g % 2}", name=f"rT{g}")
                with nc.allow_low_precision(reason="1/l broadcast tile in fp16"):
                    nc.vector.reciprocal(rT[:], pst[0:16, 0:128])
                rT1 = lt.tile([1, 2048], FP16, tag=f"rT1{g % 2}", name=f"rT1{g}")
                nc.gpsimd.dma_start(
                    rT1[0:1, :].rearrange("p (r c) -> p r c", c=128), rT[:, :])
                rT1g[g] = rT1

            def emit_R_build_b(g):
                # broadcast each 1/l row across 64 partitions via K=1 matmuls
                rT1 = rT1g.pop(g)
                psR = ps_mm.tile([128, N], FP32, tag="mm", name="mm")
                for h in range(2):
                    for it in range(8):
                        r = h * 8 + it
                        nc.tensor.matmul(
                            psR[h * 64:(h + 1) * 64, it * 128:(it + 1) * 128],
                            ones1[:, :], rT1[0:1, r * 128:(r + 1) * 128],
                            start=True, stop=True, skip_group_check=True,
                            tile_position=(0, h * 64))
                R = cxp.tile([128, N], FP16, tag=f"R{g % 2}", name=f"R{g}")
                if g % 2 == 0:
                    nc.scalar.copy(R[:], psR[:])
                else:
                    nc.vector.tensor_copy(R[:], psR[:])
                Rg[g] = R

            def emit_ctx_burst(g, j0, j1):
                if j0 == 0:
                    psc[g] = ps_mm.tile([128, N], FP32, tag="mm", name=f"c{g}")
                pc = psc[g]
                for j in range(j0, j1):
                    for lo, hi in HALVES:
                        for h in range(2):
                            base = (h * 8 + j) * N
                            nc.tensor.matmul(
                                pc[h * 64:(h + 1) * 64, lo:hi],
                                vsb[j][:, (2 * g + h) * DH:(2 * g + h + 1) * DH],
                                PT[g][:, base + lo:base + hi],
                                start=(j == 0), stop=(j == 7),
                                skip_group_check=True)

            def emit_ctx_evac(g):
                ct = cxp.tile([128, N], FP16, tag=f"cx{g}", name=f"cx{g}")
                nc.vector.tensor_mul(ct[:], psc[g][:], Rg[g][:])
                ctxT[g] = ct
                del psc[g], PT[g], PT4[g], Rg[g]

            def emit_proj(cp):
                ps = ps_mm.tile([128, N], FP32, tag="mm", name="mm")
                for t in range(KT):
                    for lo, hi in HALVES:
                        nc.tensor.matmul(ps[:, lo:hi],
                                         wpsb[t][:, cp * 128:(cp + 1) * 128],
                                         ctxT[t][:, lo:hi],
                                         start=(t == 0), stop=(t == KT - 1))
                ot = oup.tile([128, N], FP32, tag="osb", name="osb")
                nc.scalar.copy(ot[:], ps[:])
                nc.sync.dma_start(out[cp * 128:(cp + 1) * 128, :], ot[:])

            # ---- driver: software-pipelined emission
            emit_qk_block(6)
            emit_qk_block(0)
            filler = ([("qk", 7), ("qk", 1)] + [("v", j) for j in range(8)]
                      + [("qk", 8), ("qk", 2), ("qk", 9), ("qk", 3),
                         ("qk", 10), ("qk", 4), ("qk", 11), ("qk", 5)])
            for g in range(6):
                start_group(g)
                for it in range(8):
                    psA, psB = emit_S_step(g, it)
                    if filler:
                        kind, idx = filler.pop(0)
                        (emit_qk_block if kind == "qk" else emit_v_tile)(idx)
                    if g >= 1:
                        if it == 0:
                            emit_R_build_a(g - 1)
                        elif it == 1:
                            emit_ctx_burst(g - 1, 0, 4)
                        elif it == 2:
                            emit_R_build_b(g - 1)
                        elif it == 3:
                            emit_ctx_burst(g - 1, 4, 8)
                        elif it == 4:
                            emit_ctx_evac(g - 1)
                    PPt = emit_softmax(g, it, psA, psB)
                    emit_transpose(g, it, PPt)
            emit_R_build_a(5)
            emit_ctx_burst(5, 0, 4)
            emit_R_build_b(5)
            emit_ctx_burst(5, 4, 8)
            emit_ctx_evac(5)
            for cp in range(KT):
                emit_proj(cp)

    _fix_drain_waits(nc, mybir, bass_rust)
    return nc


def _prep(w_qkv, w_proj):
    r = np.arange(DIM)
    head, d = r // DH, r % DH
    qcols = d * (3 * HEADS) + 0 * HEADS + head
    kcols = d * (3 * HEADS) + 1 * HEADS + head
    vcols = d * (3 * HEADS) + 2 * HEADS + head
    w = np.asarray(w_qkv, np.float32)
    wqk = np.concatenate([w[:, qcols] * np.float32(DH ** 0.5), w[:, kcols]],
                         axis=1).astype(np.float16)
    wv = np.ascontiguousarray(w[:, vcols]).astype(np.float16)
    wp = np.asarray(w_proj, np.float32).astype(np.float16)
    return wqk, wv, wp


def _run(x, w_qkv, w_proj, **spmd_kwargs):
    import sys
    if "/opt/trn_rl_repo" not in sys.path:
        sys.path.insert(0, "/opt/trn_rl_repo")
    from concourse.bass_utils import run_bass_kernel_spmd

    if "nc" not in _cache:
        _cache["nc"] = _build()
    nc = _cache["nc"]

    x = np.asarray(x, np.float32)
    wqk, wv, wp = _prep(w_qkv, w_proj)
    xTs = x.reshape(NB, DIM, N).astype(np.float16)

    in_maps = [
        {"xT": xTs[b], "wqk": wqk, "wv": wv, "wp": wp} for b in range(NB)
    ]
    res = run_bass_kernel_spmd(nc, in_maps, list(range(NB)), **spmd_kwargs)
    outs = np.stack([np.asarray(res.results[b]["out"], np.float32)
                     for b in range(NB)])
    return outs.reshape(NB, DIM, 32, 32), res


def kernel(x, w_qkv, w_proj):
    return _run(x, w_qkv, w_proj)[0]
